# revision 15
# baseline (speedup 1.0000x reference)
"""GPT-2 transformer block on 8 trn2 NeuronCores (Bass/Tile), v2.

Sharding: token-split. Core c = 4*b + j handles batch b, output tokens
[512j, 512j+512). Host reorders each core's sequence so the own tokens sit at
positions [0,512); K/V are computed for the full (reordered) sequence, Q and
everything downstream only for positions [0,512). Causal masking:
  - non-own keys are visible to all own queries or none (per-key), applied as
    an additive bias (-1e4) inside the exp activation (per-partition bias);
  - own keys (score tiles 0..3) get a triangular mask multiply on vector.
All matmul operands are bf16 (PSUM accumulation fp32); the residual stream is
fp32. LN gains are folded into the weights on the host; LN on device is just
(x - mu) * rstd. The MLP down-projection runs in transposed layout (out.T
accumulated per E-tile) so wd streams exactly once; the host untransposes.
"""
import math
import os
import sys
import types

sys.path.insert(0, '/opt/trn_rl_repo')

import numpy as np
import ml_dtypes


def _install_ntff_shim():
    """concourse's trace path imports antenv.axon_hooks, which this image
    lacks; give it a functional stand-in so trace=True doesn't crash."""
    try:
        import antenv.axon_hooks  # noqa: F401
        return
    except ImportError:
        pass
    try:
        import antenv
    except ImportError:
        return
    mod = types.ModuleType("antenv.axon_hooks")
    mod._hook = None

    def set_axon_ntff_profile_hook(h):
        mod._hook = h

    def get_axon_ntff_profile_hook():
        return mod._hook

    mod.set_axon_ntff_profile_hook = set_axon_ntff_profile_hook
    mod.get_axon_ntff_profile_hook = get_axon_ntff_profile_hook
    sys.modules["antenv.axon_hooks"] = mod
    antenv.axon_hooks = mod
    try:
        from trn_agent_boot.trn_boot import _ntff_profile_via_ctypes
        hook = _ntff_profile_via_ctypes('/opt/axon/libaxon_pjrt.so')
        if hook is not None:
            set_axon_ntff_profile_hook(hook)
    except Exception:
        pass


_install_ntff_shim()

import concourse.bass as bass
import concourse.tile as tile
from concourse import mybir, bass_utils
from concourse.masks import make_identity

P = 128
B, S, E = 2, 2048, 2048
H, D, KH, G = 16, 128, 4, 4
F = 8192
OWN = 512                 # tokens owned per core
NE = E // P               # 16
NSK = S // P              # 16
NF = F // P               # 64
NMS = OWN // P            # 4
f32 = mybir.dt.float32
f32r = mybir.dt.float32r
bf16 = mybir.dt.bfloat16
EXP_SCALE = 1.0 / math.sqrt(D)
NEGB = -30.0              # additive key bias for hidden keys (exp->~1e-13)


def split_waits(nc, maxw=1):
    """This walrus build supports at most one sync-wait per instruction;
    hoist excess waits onto same-engine NoOps placed before the owner."""
    n = 0
    for fn in nc.m.functions:
        for blk in fn.blocks:
            new_insts = []
            for inst in blk.instructions:
                si = inst.sync_info
                if si is not None and si.on_wait and len(si.on_wait) > maxw:
                    waits = list(si.on_wait)
                    excess, keep = waits[:-maxw], waits[-maxw:]
                    for ci, w in enumerate(excess):
                        new_insts.append(mybir.InstNoOp(
                            name=f"{inst.name}-ws{ci}", engine=inst.engine,
                            sync_info=mybir.SyncInfo(on_wait=[w], on_update=[])))
                        n += 1
                    inst.sync_info = mybir.SyncInfo(
                        on_wait=keep, on_update=list(si.on_update or []))
                new_insts.append(inst)
            blk.instructions = new_insts
    return n


def _ln_tile(nc, pool, x_tile, eps_t, out_tile):
    """out = (x - mean(x)) * rsqrt(var(x) + eps) along free dim (E); bf16 out."""
    stats = pool.tile([P, E // 512, 6], f32, tag="ln_stats")
    for i in range(E // 512):
        nc.vector.bn_stats(out=stats[:, i, :], in_=x_tile[:, i * 512:(i + 1) * 512])
    mv = pool.tile([P, 2], f32, tag="ln_mv")
    nc.vector.bn_aggr(out=mv, in_=stats)
    rstd = pool.tile([P, 1], f32, tag="ln_rstd")
    nc.scalar.activation(out=rstd, in_=mv[:, 1:2],
                         func=mybir.ActivationFunctionType.Sqrt, bias=eps_t)
    nc.vector.reciprocal(out=rstd, in_=rstd)
    nc.vector.tensor_scalar(out=out_tile, in0=x_tile, scalar1=mv[:, 0:1],
                            scalar2=rstd, op0=mybir.AluOpType.subtract,
                            op1=mybir.AluOpType.mult)


def build():
    nc = bass.Bass("TRN2", target_bir_lowering=False, debug=False, num_devices=8)

    xkv = nc.dram_tensor("xkv", [S, E], bf16, kind="ExternalInput").ap()
    xres = nc.dram_tensor("xres", [OWN, E], f32, kind="ExternalInput").ap()
    tri = nc.dram_tensor("tri", [P, NMS, OWN], bf16, kind="ExternalInput").ap()
    kbias = nc.dram_tensor("kbias", [P, NSK], f32, kind="ExternalInput").ap()
    wq_s = nc.dram_tensor("wq_s", [H, P, NE, P], bf16, kind="ExternalInput").ap()
    wk_s = nc.dram_tensor("wk_s", [KH, P, NE, P], bf16, kind="ExternalInput").ap()
    wv_s = nc.dram_tensor("wv_s", [KH, P, NE, P], bf16, kind="ExternalInput").ap()
    wo_r = nc.dram_tensor("wo_r", [H, P, E], bf16, kind="ExternalInput").ap()
    wu_s = nc.dram_tensor("wu_s", [NF, P, NE, P], bf16, kind="ExternalInput").ap()
    wd_r = nc.dram_tensor("wd_r", [NF, P, E], bf16, kind="ExternalInput").ap()
    bq = nc.dram_tensor("bq", [P, H], f32, kind="ExternalInput").ap()
    bk = nc.dram_tensor("bk", [P, KH], f32, kind="ExternalInput").ap()
    bv = nc.dram_tensor("bv", [P, KH], f32, kind="ExternalInput").ap()
    bu = nc.dram_tensor("bu", [P, NF], f32, kind="ExternalInput").ap()
    bd = nc.dram_tensor("bd", [P, NE], f32, kind="ExternalInput").ap()
    outT = nc.dram_tensor("outT", [NE, P, OWN], f32, kind="ExternalOutput").ap()

    with tile.TileContext(nc) as tc:
        _build_body(nc, tc, locals())
    return nc


def _build_body(nc, tc, t_):
    xkv, xres_d, tri_d, kbias_d = t_["xkv"], t_["xres"], t_["tri"], t_["kbias"]
    wq_s, wk_s, wv_s, wo_r, wu_s, wd_r = (t_[k] for k in
                                          ("wq_s", "wk_s", "wv_s", "wo_r", "wu_s", "wd_r"))
    bq, bk, bv, bu, bd = (t_[k] for k in ("bq", "bk", "bv", "bu", "bd"))
    outT_d = t_["outT"]
    del t_
    Ident = mybir.ActivationFunctionType.Identity
    Exp = mybir.ActivationFunctionType.Exp
    Gelu = mybir.ActivationFunctionType.Gelu
    mult = mybir.AluOpType.mult
    add = mybir.AluOpType.add

    with (
        tc.tile_pool(name="persist", bufs=1) as persist,
        tc.tile_pool(name="resid", bufs=1) as resid,
    ):
        ident = persist.tile([P, P], bf16)
        make_identity(nc, ident)
        eps_t = persist.tile([P, 1], f32)
        nc.vector.memset(eps_t, 1e-5)
        ones_col = persist.tile([P, 1], bf16)   # lhsT for denominator (K=P, M=1)
        nc.vector.memset(ones_col, 1.0)
        ones_colf = persist.tile([P, 1], f32)   # f32r lhsT for the acc fold
        nc.vector.memset(ones_colf, 1.0)
        ones_row = persist.tile([1, P], f32)    # lhsT for broadcast (K=1, M=P)
        nc.vector.memset(ones_row, 1.0)
        bq_sb = persist.tile([P, H], f32)
        nc.sync.dma_start(out=bq_sb, in_=bq)
        bk_sb = persist.tile([P, KH], f32)
        nc.sync.dma_start(out=bk_sb, in_=bk)
        bv_sb = persist.tile([P, KH], f32)
        nc.sync.dma_start(out=bv_sb, in_=bv)
        kb_sb = persist.tile([P, NSK], f32)     # per-key additive exp bias
        nc.sync.dma_start(out=kb_sb, in_=kbias_d)
        tri_sb = persist.tile([P, NMS, OWN], bf16)  # triangular mask tiles

        with tc.tile_pool(name="qkv_keep", bufs=1) as qkv_keep:
            qT = [qkv_keep.tile([P, OWN], bf16, tag=f"qT{i}", name=f"qT{i}") for i in range(H)]
            kT = [qkv_keep.tile([P, S], bf16, tag=f"kT{i}", name=f"kT{i}") for i in range(KH)]
            vtok = [qkv_keep.tile([P, KH * D], bf16, tag=f"vtok{i}", name=f"vtok{i}")
                    for i in range(NSK)]
            xres = [resid.tile([P, E], f32, tag=f"xres{t}", name=f"xres{t}")
                    for t in range(NMS)]

            # ---------------- Phase 1: LN1 + Q/K/V over reordered seq --------
            with (
                tc.tile_pool(name="p1", bufs=1) as p1,
                tc.tile_pool(name="ps1", bufs=1, space="PSUM") as ps1,
            ):
                strips = [p1.tile([P, S], bf16, tag=f"x1s{e}", name=f"x1s{e}")
                          for e in range(NE)]
                for c in range(S // OWN):
                    x1c = []
                    for t in range(NMS):
                        x_t = p1.tile([P, E], bf16, tag="x_t", bufs=2)
                        nc.sync.dma_start(
                            out=x_t, in_=xkv[(4 * c + t) * P:(4 * c + t + 1) * P, :])
                        x1_t = p1.tile([P, E], bf16, tag=f"x1_{t}")
                        _ln_tile(nc, p1, x_t, eps_t, x1_t)
                        x1c.append(x1_t)
                    for e in range(NE):
                        tp4 = ps1.tile([P, NMS, P], bf16, tag="tp4", bufs=2)
                        for t in range(NMS):
                            nc.tensor.transpose(tp4[:, t, :],
                                                x1c[t][:, e * P:(e + 1) * P], ident)
                        nc.scalar.copy(strips[e][:, c * OWN:(c + 1) * OWN],
                                       tp4.rearrange("p t q -> p (t q)"))
                    # K and V projections for this chunk
                    for kv_or_v in range(2):
                        w_src, b_sb = ((wk_s, bk_sb), (wv_s, bv_sb))[kv_or_v]
                        for m in range(KH):
                            wstrip = p1.tile([P, NE, P], bf16, tag=f"w{m % 2}", bufs=2)
                            nc.sync.dma_start(out=wstrip, in_=w_src[m])
                            pskv = ps1.tile([P, OWN], f32, tag=f"ps{m % 2}", bufs=2)
                            for e in range(NE):
                                nc.tensor.matmul(pskv, wstrip[:, e, :],
                                                 strips[e][:, c * OWN:(c + 1) * OWN],
                                                 start=(e == 0), stop=(e == NE - 1))
                            if kv_or_v == 0:
                                nc.scalar.activation(
                                    out=kT[m][:, c * OWN:(c + 1) * OWN],
                                    in_=pskv, func=Ident, bias=b_sb[:, m:m + 1])
                            else:
                                vf = p1.tile([P, OWN], bf16, tag=f"vf{m % 2}", bufs=2)
                                nc.scalar.activation(out=vf, in_=pskv, func=Ident,
                                                     bias=b_sb[:, m:m + 1])
                                tpv = ps1.tile([P, NMS, P], bf16, tag="tpv", bufs=2)
                                for t in range(NMS):
                                    nc.tensor.transpose(
                                        tpv[:, t, :], vf[:, t * P:(t + 1) * P], ident)
                                for t in range(NMS):
                                    nc.scalar.copy(
                                        vtok[c * NMS + t][:, m * P:(m + 1) * P],
                                        tpv[:, t, :])
                    if c == 0:
                        # Q projections for own tokens (positions [0, 512))
                        for m in range(H):
                            wstrip = p1.tile([P, NE, P], bf16, tag=f"w{m % 2}", bufs=2)
                            nc.sync.dma_start(out=wstrip, in_=wq_s[m])
                            psq = ps1.tile([P, OWN], f32, tag=f"ps{m % 2}", bufs=2)
                            for e in range(NE):
                                nc.tensor.matmul(psq, wstrip[:, e, :],
                                                 strips[e][:, 0:OWN],
                                                 start=(e == 0), stop=(e == NE - 1))
                            nc.scalar.activation(out=qT[m], in_=psq, func=Ident,
                                                 bias=bq_sb[:, m:m + 1])

            # ---------------- Phase 2: attention -> oT -----------------------
            with tc.tile_pool(name="oT_keep", bufs=1) as oT_keep:
                oT = [oT_keep.tile([P, OWN], bf16, tag=f"oT{i}", name=f"oT{i}")
                      for i in range(H)]
                with (
                    tc.tile_pool(name="p2", bufs=1) as p2,
                    tc.tile_pool(name="ps2", bufs=1, space="PSUM") as ps2,
                ):
                    nc.sync.dma_start(out=tri_sb, in_=tri_d)
                    for t in range(NMS):
                        nc.sync.dma_start(out=xres[t], in_=xres_d[t * P:(t + 1) * P, :])
                    for h in range(H):
                        kv = h // G
                        ps_o = ps2.tile([P, OWN], f32, tag="ps_o", bufs=2)
                        ps_den = ps2.tile([1, OWN], f32, tag="ps_den", bufs=1)
                        acc = p2.tile([P, OWN], f32r, tag="acc", bufs=2)
                        for pr in range(NSK // 2):
                            sk0, sk1 = 2 * pr, 2 * pr + 1
                            ps_s = ps2.tile([P, 2, OWN], f32, tag="ps_s", bufs=2)
                            for i in (0, 1):
                                nc.tensor.matmul(
                                    ps_s[:, i, :],
                                    kT[kv][:, (sk0 + i) * P:(sk0 + i + 1) * P],
                                    qT[h], start=True, stop=True)
                            exr = p2.tile([P, 2, OWN], bf16, tag="exr", bufs=3)
                            nc.scalar.activation(
                                out=exr.rearrange("p t q -> p (t q)"),
                                in_=ps_s.rearrange("p t q -> p (t q)"), func=Exp,
                                scale=EXP_SCALE, bias=kb_sb[:, sk0:sk0 + 1])
                            if sk0 < NMS:  # diagonal: triangular mask multiply
                                nc.vector.tensor_tensor(
                                    out=exr.rearrange("p t q -> p (t q)"),
                                    in0=exr.rearrange("p t q -> p (t q)"),
                                    in1=tri_sb[:, sk0:sk0 + 2, :].rearrange(
                                        "p t q -> p (t q)"), op=mult)
                            for i in (0, 1):
                                nc.tensor.matmul(ps_o,
                                                 vtok[sk0 + i][:, kv * P:(kv + 1) * P],
                                                 exr[:, i, :], start=(sk0 + i == 0),
                                                 stop=(sk0 + i == NSK - 1))
                            # denominator in f32: even tiles on PE (PSUM),
                            # odd tiles accumulated on vector, folded below
                            nc.tensor.matmul(ps_den, ones_col, exr[:, 0, :],
                                             start=(pr == 0), stop=False)
                            if pr == 0:
                                with nc.allow_low_precision(reason="softmax den"):
                                    nc.vector.tensor_copy(acc, exr[:, 1, :])
                            else:
                                with nc.allow_low_precision(reason="softmax den"):
                                    nc.vector.tensor_tensor(out=acc, in0=acc,
                                                            in1=exr[:, 1, :], op=add)
                        nc.tensor.matmul(ps_den, ones_colf.bitcast(f32r),
                                         acc, start=False, stop=True)
                        rden = p2.tile([1, OWN], f32r, tag="rden", bufs=2)
                        with nc.allow_low_precision(reason="softmax denominator"):
                            nc.vector.reciprocal(out=rden, in_=ps_den)
                        ps_bc = ps2.tile([P, OWN], f32, tag="ps_bc", bufs=1)
                        nc.tensor.matmul(ps_bc, ones_row.bitcast(f32r), rden,
                                         start=True, stop=True)
                        bc = p2.tile([P, OWN], f32, tag="bc", bufs=2)
                        nc.vector.tensor_copy(bc, ps_bc)
                        nc.vector.tensor_tensor(out=oT[h], in0=ps_o, in1=bc, op=mult)

                # ---------------- Phase 3: o-proj + residual -> xres ---------
                with (
                    tc.tile_pool(name="p3", bufs=1) as p3,
                    tc.tile_pool(name="ps3", bufs=1, space="PSUM") as ps3,
                ):
                    for eh in range(2):
                        pso = [ps3.tile([P, OWN], f32, tag=f"pso{i}", bufs=1,
                                        name=f"pso{i}") for i in range(8)]
                        for k in range(H):
                            wtile = p3.tile([P, E], bf16, tag="wo", bufs=3)
                            nc.sync.dma_start(out=wtile, in_=wo_r[k])
                            for ec in range(2):
                                for ms in range(NMS):
                                    nc.tensor.matmul(
                                        pso[ms * 2 + ec],
                                        oT[k][:, ms * P:(ms + 1) * P],
                                        wtile[:, (2 * eh + ec) * OWN:(2 * eh + ec + 1) * OWN],
                                        start=(k == 0), stop=(k == H - 1))
                        for ms in range(NMS):
                            for ec in range(2):
                                lo = (2 * eh + ec) * OWN
                                nc.vector.tensor_tensor(
                                    out=xres[ms][:, lo:lo + OWN],
                                    in0=pso[ms * 2 + ec],
                                    in1=xres[ms][:, lo:lo + OWN], op=add)

        # ---------------- Phase 4: LN2 -> x2T strips + xresT ----------------
        with tc.tile_pool(name="mlp_keep", bufs=1) as mlp_keep:
            x2T = [mlp_keep.tile([P, OWN], bf16, tag=f"x2T{e}", name=f"x2T{e}")
                   for e in range(NE)]
            xresT = [mlp_keep.tile([P, OWN], bf16, tag=f"xrT{e}", name=f"xrT{e}")
                     for e in range(NE)]
            hT = [mlp_keep.tile([P, OWN], bf16, tag=f"hT{i}", name=f"hT{i}")
                  for i in range(NF)]

            with (
                tc.tile_pool(name="p4", bufs=1) as p4,
                tc.tile_pool(name="ps4", bufs=1, space="PSUM") as ps4,
            ):
                x2 = []
                xrb = []
                for t in range(NMS):
                    x2_t = p4.tile([P, E], bf16, tag=f"x2_{t}", name=f"x2_{t}")
                    _ln_tile(nc, p4, xres[t], eps_t, x2_t)
                    x2.append(x2_t)
                    xr_t = p4.tile([P, E], bf16, tag=f"xrb{t}", name=f"xrb{t}")
                    nc.vector.tensor_copy(xr_t, xres[t])
                    xrb.append(xr_t)
                for e in range(NE):
                    tp4 = ps4.tile([P, NMS, P], bf16, tag="tp4", bufs=2)
                    for t in range(NMS):
                        nc.tensor.transpose(tp4[:, t, :], x2[t][:, e * P:(e + 1) * P],
                                            ident)
                    nc.scalar.copy(x2T[e], tp4.rearrange("p t q -> p (t q)"))
                    tpr = ps4.tile([P, NMS, P], bf16, tag="tpr", bufs=2)
                    for t in range(NMS):
                        nc.tensor.transpose(tpr[:, t, :], xrb[t][:, e * P:(e + 1) * P],
                                            ident)
                    nc.scalar.copy(xresT[e], tpr.rearrange("p t q -> p (t q)"))

            # ---------------- Phase 5: MLP up (gelu) -> hT ------------------
            with (
                tc.tile_pool(name="p5", bufs=1) as p5,
                tc.tile_pool(name="ps5", bufs=1, space="PSUM") as ps5,
            ):
                bu_sb = p5.tile([P, NF], f32)
                nc.sync.dma_start(out=bu_sb, in_=bu)
                for f in range(NF):
                    wstrip = p5.tile([P, NE, P], bf16, tag=f"wu{f % 2}", bufs=2)
                    nc.sync.dma_start(out=wstrip, in_=wu_s[f])
                    psh = ps5.tile([P, OWN], f32, tag=f"psh{f % 2}", bufs=2)
                    for e in range(NE):
                        nc.tensor.matmul(psh, wstrip[:, e, :], x2T[e],
                                         start=(e == 0), stop=(e == NE - 1))
                    nc.scalar.activation(out=hT[f], in_=psh, func=Gelu,
                                         bias=bu_sb[:, f:f + 1])

            # ------------- Phase 6: MLP down (transposed) + residual --------
            with (
                tc.tile_pool(name="p6", bufs=1) as p6,
                tc.tile_pool(name="ps6", bufs=1, space="PSUM") as ps6,
            ):
                bd_sb = p6.tile([P, NE], f32)
                nc.sync.dma_start(out=bd_sb, in_=bd)
                for eg in range(2):
                    psd = [ps6.tile([P, OWN], f32, tag=f"psd{i}", bufs=1,
                                    name=f"psd{i}") for i in range(8)]
                    for fi in range(NF):
                        wtile = p6.tile([P, 8 * P], bf16, tag=f"wd{fi % 2}", bufs=3)
                        nc.sync.dma_start(
                            out=wtile, in_=wd_r[fi][:, eg * 8 * P:(eg + 1) * 8 * P])
                        for i in range(8):
                            nc.tensor.matmul(psd[i], wtile[:, i * P:(i + 1) * P],
                                             hT[fi], start=(fi == 0),
                                             stop=(fi == NF - 1))
                    for i in range(8):
                        e = eg * 8 + i
                        ot = p6.tile([P, OWN], f32, tag="ot", bufs=3)
                        nc.vector.tensor_tensor(out=ot, in0=psd[i], in1=xresT[e],
                                                op=add)
                        nc.vector.tensor_scalar(
                            out=ot, in0=ot, scalar1=bd_sb[:, e:e + 1], scalar2=None,
                            op0=add)
                        nc.sync.dma_start(out=outT_d[e], in_=ot)


_NC_CACHE = None
LAST_RESULTS = None


def _get_nc():
    global _NC_CACHE
    if _NC_CACHE is None:
        nc = build()
        split_waits(nc)
        _NC_CACHE = nc
    return _NC_CACHE


def _prep_shared(ln1_g, ln1_b, wq, bq, wk, bk, wv, bv, wo, bo, ln2_g, ln2_b,
                 wu, bu, wd, bd):
    f = np.float64
    ln1_g, ln1_b = np.asarray(ln1_g, f), np.asarray(ln1_b, f)
    ln2_g, ln2_b = np.asarray(ln2_g, f), np.asarray(ln2_b, f)
    wq, wk, wv = np.asarray(wq, f), np.asarray(wk, f), np.asarray(wv, f)
    wo, wu, wd = np.asarray(wo, f), np.asarray(wu, f), np.asarray(wd, f)
    # fold LN gains into weights, LN biases into projection biases
    wq_f, bq_f = ln1_g[:, None] * wq, np.asarray(bq, f) + ln1_b @ wq
    wk_f, bk_f = ln1_g[:, None] * wk, np.asarray(bk, f) + ln1_b @ wk
    wv_f, bv_f = ln1_g[:, None] * wv, np.asarray(bv, f) + ln1_b @ wv
    wu_f, bu_f = ln2_g[:, None] * wu, np.asarray(bu, f) + ln2_b @ wu

    def strips(w, n):  # [E, n*128] -> [n, 128(p), NE, 128(m)] contiguous DMA
        return np.ascontiguousarray(
            w.reshape(NE, P, n, P).transpose(2, 1, 0, 3)).astype(ml_dtypes.bfloat16)

    def rows(w, nr):   # [nr*128, E] -> [nr, 128, E]
        return np.ascontiguousarray(w.reshape(nr, P, E)).astype(ml_dtypes.bfloat16)

    tri = np.triu(np.ones((OWN, OWN), np.float32))  # [key, query]: k <= q
    tri = np.ascontiguousarray(
        tri.reshape(NMS, P, OWN).transpose(1, 0, 2)).astype(ml_dtypes.bfloat16)

    def ptile(v, n):  # [n*128] -> [128, n] (partition-major)
        return np.ascontiguousarray(
            np.asarray(v).reshape(n, P).T).astype(np.float32)

    return {
        "wq_s": strips(wq_f, H), "wk_s": strips(wk_f, KH), "wv_s": strips(wv_f, KH),
        "wo_r": rows(wo, H), "wu_s": strips(wu_f, NF), "wd_r": rows(wd, NF),
        "bq": ptile(bq_f, H), "bk": ptile(bk_f, KH),
        "bv": ptile(bv_f, KH), "bu": ptile(bu_f, NF),
        "bd": ptile(np.asarray(bd, f), NE), "tri": tri,
    }, np.asarray(bo, f)


def kernel(x, ln1_g, ln1_b, wq, bq, wk, bk, wv, bv, wo, bo, ln2_g, ln2_b,
           wu, bu, wd, bd):
    x = np.asarray(x, np.float32)
    shared, bo_f = _prep_shared(ln1_g, ln1_b, wq, bq, wk, bk, wv, bv, wo, bo,
                                ln2_g, ln2_b, wu, bu, wd, bd)
    in_maps = []
    for core in range(8):
        b, j = divmod(core, 4)
        m = dict(shared)
        own = slice(OWN * j, OWN * (j + 1))
        # reorder: own tokens first, then the rest in natural order
        order = np.concatenate([np.arange(OWN * j, OWN * (j + 1)),
                                np.arange(0, OWN * j),
                                np.arange(OWN * (j + 1), S)])
        m["xkv"] = np.ascontiguousarray(x[b][order]).astype(ml_dtypes.bfloat16)
        m["xres"] = np.ascontiguousarray(x[b, own] + bo_f[None, :]).astype(np.float32)
        # per-key additive bias: 0 if key visible to all own queries (or own),
        # NEGB if hidden from all own queries
        kb = np.where(order < OWN * (j + 1), 0.0, NEGB).astype(np.float32)
        m["kbias"] = np.ascontiguousarray(kb.reshape(NSK, P).T).astype(np.float32)
        in_maps.append(m)

    nc = _get_nc()
    trace = bool(os.environ.get("KERNEL_TRACE"))
    res = bass_utils.run_bass_kernel_spmd(
        nc, in_maps, core_ids=list(range(8)), trace=trace)
    global LAST_RESULTS
    LAST_RESULTS = res
    out = np.empty((B, S, E), np.float32)
    for core in range(8):
        b, j = divmod(core, 4)
        # outT is [NE, 128, OWN] = out[own].T tiled; untranspose on host
        oT = res.results[core]["outT"]
        out[b, OWN * j:OWN * (j + 1)] = oT.transpose(2, 0, 1).reshape(OWN, E)
    return out


# revision 16
# speedup vs baseline: 1.2509x; 1.2509x over previous
"""GPT-2 transformer block on 8 trn2 NeuronCores (Bass/Tile), v2.

Sharding: token-split. Core c = 4*b + j handles batch b, output tokens
[512j, 512j+512). Host reorders each core's sequence so the own tokens sit at
positions [0,512); K/V are computed for the full (reordered) sequence, Q and
everything downstream only for positions [0,512). Causal masking:
  - non-own keys are visible to all own queries or none (per-key), applied as
    an additive bias (-1e4) inside the exp activation (per-partition bias);
  - own keys (score tiles 0..3) get a triangular mask multiply on vector.
All matmul operands are bf16 (PSUM accumulation fp32); the residual stream is
fp32. LN gains are folded into the weights on the host; LN on device is just
(x - mu) * rstd. The MLP down-projection runs in transposed layout (out.T
accumulated per E-tile) so wd streams exactly once; the host untransposes.
"""
import math
import os
import sys
import types

sys.path.insert(0, '/opt/trn_rl_repo')

import numpy as np
import ml_dtypes


def _install_ntff_shim():
    """concourse's trace path imports antenv.axon_hooks, which this image
    lacks; give it a functional stand-in so trace=True doesn't crash."""
    try:
        import antenv.axon_hooks  # noqa: F401
        return
    except ImportError:
        pass
    try:
        import antenv
    except ImportError:
        return
    mod = types.ModuleType("antenv.axon_hooks")
    mod._hook = None

    def set_axon_ntff_profile_hook(h):
        mod._hook = h

    def get_axon_ntff_profile_hook():
        return mod._hook

    mod.set_axon_ntff_profile_hook = set_axon_ntff_profile_hook
    mod.get_axon_ntff_profile_hook = get_axon_ntff_profile_hook
    sys.modules["antenv.axon_hooks"] = mod
    antenv.axon_hooks = mod
    try:
        from trn_agent_boot.trn_boot import _ntff_profile_via_ctypes
        hook = _ntff_profile_via_ctypes('/opt/axon/libaxon_pjrt.so')
        if hook is not None:
            set_axon_ntff_profile_hook(hook)
    except Exception:
        pass


_install_ntff_shim()

import concourse.bass as bass
import concourse.tile as tile
from concourse import mybir, bass_utils
from concourse.masks import make_identity

P = 128
B, S, E = 2, 2048, 2048
H, D, KH, G = 16, 128, 4, 4
F = 8192
OWN = 512                 # tokens owned per core
NE = E // P               # 16
NSK = S // P              # 16
NF = F // P               # 64
NMS = OWN // P            # 4
f32 = mybir.dt.float32
f32r = mybir.dt.float32r
bf16 = mybir.dt.bfloat16
EXP_SCALE = 1.0 / math.sqrt(D)
NEGB = -30.0              # additive key bias for hidden keys (exp->~1e-13)


def split_waits(nc, maxw=1):
    """This walrus build supports at most one sync-wait per instruction;
    hoist excess waits onto same-engine NoOps placed before the owner."""
    n = 0
    for fn in nc.m.functions:
        for blk in fn.blocks:
            new_insts = []
            for inst in blk.instructions:
                si = inst.sync_info
                if si is not None and si.on_wait and len(si.on_wait) > maxw:
                    waits = list(si.on_wait)
                    excess, keep = waits[:-maxw], waits[-maxw:]
                    for ci, w in enumerate(excess):
                        new_insts.append(mybir.InstNoOp(
                            name=f"{inst.name}-ws{ci}", engine=inst.engine,
                            sync_info=mybir.SyncInfo(on_wait=[w], on_update=[])))
                        n += 1
                    inst.sync_info = mybir.SyncInfo(
                        on_wait=keep, on_update=list(si.on_update or []))
                new_insts.append(inst)
            blk.instructions = new_insts
    return n


def _ln_tile(nc, pool, x_tile, eps_t, out_tile):
    """out = (x - mean(x)) * rsqrt(var(x) + eps) along free dim (E); bf16 out."""
    stats = pool.tile([P, E // 512, 6], f32, tag="ln_stats")
    for i in range(E // 512):
        nc.vector.bn_stats(out=stats[:, i, :], in_=x_tile[:, i * 512:(i + 1) * 512])
    mv = pool.tile([P, 2], f32, tag="ln_mv")
    nc.vector.bn_aggr(out=mv, in_=stats)
    rstd = pool.tile([P, 1], f32, tag="ln_rstd")
    nc.scalar.activation(out=rstd, in_=mv[:, 1:2],
                         func=mybir.ActivationFunctionType.Sqrt, bias=eps_t)
    nc.vector.reciprocal(out=rstd, in_=rstd)
    nc.vector.tensor_scalar(out=out_tile, in0=x_tile, scalar1=mv[:, 0:1],
                            scalar2=rstd, op0=mybir.AluOpType.subtract,
                            op1=mybir.AluOpType.mult)


def build():
    nc = bass.Bass("TRN2", target_bir_lowering=False, debug=False, num_devices=8)

    xkv = nc.dram_tensor("xkv", [S, E], bf16, kind="ExternalInput").ap()
    xres = nc.dram_tensor("xres", [OWN, E], f32, kind="ExternalInput").ap()
    tri = nc.dram_tensor("tri", [P, NMS, OWN], bf16, kind="ExternalInput").ap()
    kbias = nc.dram_tensor("kbias", [P, NSK], f32, kind="ExternalInput").ap()
    wq_s = nc.dram_tensor("wq_s", [H, P, NE, P], bf16, kind="ExternalInput").ap()
    wk_s = nc.dram_tensor("wk_s", [KH, P, NE, P], bf16, kind="ExternalInput").ap()
    wv_s = nc.dram_tensor("wv_s", [KH, P, NE, P], bf16, kind="ExternalInput").ap()
    wo_r = nc.dram_tensor("wo_r", [H, P, E], bf16, kind="ExternalInput").ap()
    wu_s = nc.dram_tensor("wu_s", [NF, P, NE, P], bf16, kind="ExternalInput").ap()
    wd_r = nc.dram_tensor("wd_r", [NF, P, E], bf16, kind="ExternalInput").ap()
    bq = nc.dram_tensor("bq", [P, H], f32, kind="ExternalInput").ap()
    bk = nc.dram_tensor("bk", [P, KH], f32, kind="ExternalInput").ap()
    bv = nc.dram_tensor("bv", [P, KH], f32, kind="ExternalInput").ap()
    bu = nc.dram_tensor("bu", [P, NF], f32, kind="ExternalInput").ap()
    bd = nc.dram_tensor("bd", [P, NE], f32, kind="ExternalInput").ap()
    outT = nc.dram_tensor("outT", [NE, P, OWN], f32, kind="ExternalOutput").ap()

    with tile.TileContext(nc) as tc:
        _build_body(nc, tc, locals())
    return nc


def _build_body(nc, tc, t_):
    xkv, xres_d, tri_d, kbias_d = t_["xkv"], t_["xres"], t_["tri"], t_["kbias"]
    wq_s, wk_s, wv_s, wo_r, wu_s, wd_r = (t_[k] for k in
                                          ("wq_s", "wk_s", "wv_s", "wo_r", "wu_s", "wd_r"))
    bq, bk, bv, bu, bd = (t_[k] for k in ("bq", "bk", "bv", "bu", "bd"))
    outT_d = t_["outT"]
    del t_
    Ident = mybir.ActivationFunctionType.Identity
    Exp = mybir.ActivationFunctionType.Exp
    Gelu = mybir.ActivationFunctionType.Gelu
    mult = mybir.AluOpType.mult
    add = mybir.AluOpType.add

    with (
        tc.tile_pool(name="persist", bufs=1) as persist,
        tc.tile_pool(name="resid", bufs=1) as resid,
    ):
        ident = persist.tile([P, P], bf16)
        make_identity(nc, ident)
        eps_t = persist.tile([P, 1], f32)
        nc.vector.memset(eps_t, 1e-5)
        ones_col = persist.tile([P, 1], bf16)   # lhsT for denominator (K=P, M=1)
        nc.vector.memset(ones_col, 1.0)
        ones_colf = persist.tile([P, 1], f32)   # f32r lhsT for the acc fold
        nc.vector.memset(ones_colf, 1.0)
        ones_row = persist.tile([1, P], f32)    # lhsT for broadcast (K=1, M=P)
        nc.vector.memset(ones_row, 1.0)
        bq_sb = persist.tile([P, H], f32)
        nc.sync.dma_start(out=bq_sb, in_=bq)
        bk_sb = persist.tile([P, KH], f32)
        nc.sync.dma_start(out=bk_sb, in_=bk)
        bv_sb = persist.tile([P, KH], f32)
        nc.sync.dma_start(out=bv_sb, in_=bv)
        kb_sb = persist.tile([P, NSK], f32)     # per-key additive exp bias
        nc.sync.dma_start(out=kb_sb, in_=kbias_d)
        tri_sb = persist.tile([P, NMS, OWN], bf16)  # triangular mask tiles

        with tc.tile_pool(name="qkv_keep", bufs=1) as qkv_keep:
            qT = [qkv_keep.tile([P, OWN], bf16, tag=f"qT{i}", name=f"qT{i}") for i in range(H)]
            kT = [qkv_keep.tile([P, S], bf16, tag=f"kT{i}", name=f"kT{i}") for i in range(KH)]
            vtok = [qkv_keep.tile([P, KH * D], bf16, tag=f"vtok{i}", name=f"vtok{i}")
                    for i in range(NSK)]
            xres = [resid.tile([P, E], f32, tag=f"xres{t}", name=f"xres{t}")
                    for t in range(NMS)]

            # ---------------- Phase 1: LN1 + Q/K/V over reordered seq --------
            with (
                tc.tile_pool(name="p1", bufs=1) as p1,
                tc.tile_pool(name="ps1", bufs=1, space="PSUM") as ps1,
            ):
                strips = [p1.tile([P, S], bf16, tag=f"x1s{e}", name=f"x1s{e}")
                          for e in range(NE)]
                for c in range(S // OWN):
                    x1c = []
                    for t in range(NMS):
                        x_t = p1.tile([P, E], bf16, tag="x_t", bufs=2)
                        nc.sync.dma_start(
                            out=x_t, in_=xkv[(4 * c + t) * P:(4 * c + t + 1) * P, :])
                        x1_t = p1.tile([P, E], bf16, tag=f"x1_{t}")
                        _ln_tile(nc, p1, x_t, eps_t, x1_t)
                        x1c.append(x1_t)
                    for e in range(NE):
                        tp4 = ps1.tile([P, NMS, P], bf16, tag="tp4", bufs=2)
                        for t in range(NMS):
                            nc.tensor.transpose(tp4[:, t, :],
                                                x1c[t][:, e * P:(e + 1) * P], ident)
                        nc.scalar.copy(strips[e][:, c * OWN:(c + 1) * OWN],
                                       tp4.rearrange("p t q -> p (t q)"))
                    # K and V projections for this chunk
                    for kv_or_v in range(2):
                        w_src, b_sb = ((wk_s, bk_sb), (wv_s, bv_sb))[kv_or_v]
                        for m in range(KH):
                            wstrip = p1.tile([P, NE, P], bf16, tag=f"w{m % 2}", bufs=2)
                            nc.sync.dma_start(out=wstrip, in_=w_src[m])
                            pskv = ps1.tile([P, OWN], f32, tag=f"ps{m % 2}", bufs=2)
                            for e in range(NE):
                                nc.tensor.matmul(pskv, wstrip[:, e, :],
                                                 strips[e][:, c * OWN:(c + 1) * OWN],
                                                 start=(e == 0), stop=(e == NE - 1))
                            if kv_or_v == 0:
                                nc.scalar.activation(
                                    out=kT[m][:, c * OWN:(c + 1) * OWN],
                                    in_=pskv, func=Ident, bias=b_sb[:, m:m + 1])
                            else:
                                vf = p1.tile([P, OWN], bf16, tag=f"vf{m % 2}", bufs=2)
                                nc.scalar.activation(out=vf, in_=pskv, func=Ident,
                                                     bias=b_sb[:, m:m + 1])
                                tpv = ps1.tile([P, NMS, P], bf16, tag="tpv", bufs=2)
                                for t in range(NMS):
                                    nc.tensor.transpose(
                                        tpv[:, t, :], vf[:, t * P:(t + 1) * P], ident)
                                for t in range(NMS):
                                    nc.scalar.copy(
                                        vtok[c * NMS + t][:, m * P:(m + 1) * P],
                                        tpv[:, t, :])
                    if c == 0:
                        # Q projections for own tokens (positions [0, 512))
                        for m in range(H):
                            wstrip = p1.tile([P, NE, P], bf16, tag=f"w{m % 2}", bufs=2)
                            nc.sync.dma_start(out=wstrip, in_=wq_s[m])
                            psq = ps1.tile([P, OWN], f32, tag=f"ps{m % 2}", bufs=2)
                            for e in range(NE):
                                nc.tensor.matmul(psq, wstrip[:, e, :],
                                                 strips[e][:, 0:OWN],
                                                 start=(e == 0), stop=(e == NE - 1))
                            nc.scalar.activation(out=qT[m], in_=psq, func=Ident,
                                                 bias=bq_sb[:, m:m + 1])

            # ---------------- Phase 2: attention -> oT -----------------------
            with tc.tile_pool(name="oT_keep", bufs=1) as oT_keep:
                oT = [oT_keep.tile([P, OWN], bf16, tag=f"oT{i}", name=f"oT{i}")
                      for i in range(H)]
                with (
                    tc.tile_pool(name="p2", bufs=1) as p2,
                    tc.tile_pool(name="ps2", bufs=1, space="PSUM") as ps2,
                ):
                    nc.sync.dma_start(out=tri_sb, in_=tri_d)
                    for t in range(NMS):
                        nc.sync.dma_start(out=xres[t], in_=xres_d[t * P:(t + 1) * P, :])
                    for h in range(H):
                        kv = h // G
                        ps_o = ps2.tile([P, OWN], f32, tag="ps_o", bufs=2)
                        ps_den = ps2.tile([1, OWN], f32, tag="ps_den", bufs=2)
                        acc = p2.tile([P, OWN], f32r, tag="acc", bufs=2)
                        for sk in range(NSK):
                            ps_s = ps2.tile([P, OWN], f32, tag="ps_s", bufs=3)
                            nc.tensor.matmul(ps_s, kT[kv][:, sk * P:(sk + 1) * P],
                                             qT[h], start=True, stop=True)
                            exr = p2.tile([P, OWN], bf16, tag="exr", bufs=4)
                            nc.scalar.activation(out=exr, in_=ps_s, func=Exp,
                                                 scale=EXP_SCALE,
                                                 bias=kb_sb[:, sk:sk + 1])
                            if sk < NMS:  # diagonal: triangular mask multiply
                                nc.vector.tensor_tensor(out=exr, in0=exr,
                                                        in1=tri_sb[:, sk, :], op=mult)
                            nc.tensor.matmul(ps_o, vtok[sk][:, kv * P:(kv + 1) * P],
                                             exr, start=(sk == 0), stop=(sk == NSK - 1))
                            # denominator in f32: even tiles on PE (PSUM),
                            # odd tiles accumulated on vector, folded below
                            if sk % 2 == 0:
                                nc.tensor.matmul(ps_den, ones_col, exr,
                                                 start=(sk == 0), stop=False)
                            elif sk == 1:
                                with nc.allow_low_precision(reason="softmax den"):
                                    nc.vector.tensor_copy(acc, exr)
                            else:
                                with nc.allow_low_precision(reason="softmax den"):
                                    nc.vector.tensor_tensor(out=acc, in0=acc,
                                                            in1=exr, op=add)
                        nc.tensor.matmul(ps_den, ones_colf.bitcast(f32r),
                                         acc, start=False, stop=True)
                        rden = p2.tile([1, OWN], f32r, tag="rden", bufs=2)
                        with nc.allow_low_precision(reason="softmax denominator"):
                            nc.vector.reciprocal(out=rden, in_=ps_den)
                        ps_bc = ps2.tile([P, OWN], f32, tag="ps_bc", bufs=1)
                        nc.tensor.matmul(ps_bc, ones_row.bitcast(f32r), rden,
                                         start=True, stop=True)
                        bc = p2.tile([P, OWN], f32, tag="bc", bufs=2)
                        nc.vector.tensor_copy(bc, ps_bc)
                        nc.vector.tensor_tensor(out=oT[h], in0=ps_o, in1=bc, op=mult)

                # ---------------- Phase 3: o-proj + residual -> xres ---------
                with (
                    tc.tile_pool(name="p3", bufs=1) as p3,
                    tc.tile_pool(name="ps3", bufs=1, space="PSUM") as ps3,
                ):
                    for eh in range(2):
                        pso = [ps3.tile([P, OWN], f32, tag=f"pso{i}", bufs=1,
                                        name=f"pso{i}") for i in range(8)]
                        for k in range(H):
                            wtile = p3.tile([P, E], bf16, tag="wo", bufs=3)
                            nc.sync.dma_start(out=wtile, in_=wo_r[k])
                            for ec in range(2):
                                for ms in range(NMS):
                                    nc.tensor.matmul(
                                        pso[ms * 2 + ec],
                                        oT[k][:, ms * P:(ms + 1) * P],
                                        wtile[:, (2 * eh + ec) * OWN:(2 * eh + ec + 1) * OWN],
                                        start=(k == 0), stop=(k == H - 1))
                        for ms in range(NMS):
                            for ec in range(2):
                                lo = (2 * eh + ec) * OWN
                                nc.vector.tensor_tensor(
                                    out=xres[ms][:, lo:lo + OWN],
                                    in0=pso[ms * 2 + ec],
                                    in1=xres[ms][:, lo:lo + OWN], op=add)

        # ---------------- Phase 4: LN2 -> x2T strips + xresT ----------------
        with tc.tile_pool(name="mlp_keep", bufs=1) as mlp_keep:
            x2T = [mlp_keep.tile([P, OWN], bf16, tag=f"x2T{e}", name=f"x2T{e}")
                   for e in range(NE)]
            xresT = [mlp_keep.tile([P, OWN], bf16, tag=f"xrT{e}", name=f"xrT{e}")
                     for e in range(NE)]
            hT = [mlp_keep.tile([P, OWN], bf16, tag=f"hT{i}", name=f"hT{i}")
                  for i in range(NF)]

            with (
                tc.tile_pool(name="p4", bufs=1) as p4,
                tc.tile_pool(name="ps4", bufs=1, space="PSUM") as ps4,
            ):
                x2 = []
                xrb = []
                for t in range(NMS):
                    x2_t = p4.tile([P, E], bf16, tag=f"x2_{t}", name=f"x2_{t}")
                    _ln_tile(nc, p4, xres[t], eps_t, x2_t)
                    x2.append(x2_t)
                    xr_t = p4.tile([P, E], bf16, tag=f"xrb{t}", name=f"xrb{t}")
                    nc.vector.tensor_copy(xr_t, xres[t])
                    xrb.append(xr_t)
                for e in range(NE):
                    tp4 = ps4.tile([P, NMS, P], bf16, tag="tp4", bufs=2)
                    for t in range(NMS):
                        nc.tensor.transpose(tp4[:, t, :], x2[t][:, e * P:(e + 1) * P],
                                            ident)
                    nc.scalar.copy(x2T[e], tp4.rearrange("p t q -> p (t q)"))
                    tpr = ps4.tile([P, NMS, P], bf16, tag="tpr", bufs=2)
                    for t in range(NMS):
                        nc.tensor.transpose(tpr[:, t, :], xrb[t][:, e * P:(e + 1) * P],
                                            ident)
                    nc.scalar.copy(xresT[e], tpr.rearrange("p t q -> p (t q)"))

            # ---------------- Phase 5: MLP up (gelu) -> hT ------------------
            with (
                tc.tile_pool(name="p5", bufs=1) as p5,
                tc.tile_pool(name="ps5", bufs=1, space="PSUM") as ps5,
            ):
                bu_sb = p5.tile([P, NF], f32)
                nc.sync.dma_start(out=bu_sb, in_=bu)
                for f in range(NF):
                    wstrip = p5.tile([P, NE, P], bf16, tag=f"wu{f % 2}", bufs=2)
                    nc.sync.dma_start(out=wstrip, in_=wu_s[f])
                    psh = ps5.tile([P, OWN], f32, tag=f"psh{f % 2}", bufs=2)
                    for e in range(NE):
                        nc.tensor.matmul(psh, wstrip[:, e, :], x2T[e],
                                         start=(e == 0), stop=(e == NE - 1))
                    nc.scalar.activation(out=hT[f], in_=psh, func=Gelu,
                                         bias=bu_sb[:, f:f + 1])

            # ------------- Phase 6: MLP down (transposed) + residual --------
            with (
                tc.tile_pool(name="p6", bufs=1) as p6,
                tc.tile_pool(name="ps6", bufs=1, space="PSUM") as ps6,
            ):
                bd_sb = p6.tile([P, NE], f32)
                nc.sync.dma_start(out=bd_sb, in_=bd)
                for eg in range(2):
                    psd = [ps6.tile([P, OWN], f32, tag=f"psd{i}", bufs=1,
                                    name=f"psd{i}") for i in range(8)]
                    for fi in range(NF):
                        wtile = p6.tile([P, 8 * P], bf16, tag=f"wd{fi % 2}", bufs=3)
                        nc.sync.dma_start(
                            out=wtile, in_=wd_r[fi][:, eg * 8 * P:(eg + 1) * 8 * P])
                        for i in range(8):
                            nc.tensor.matmul(psd[i], wtile[:, i * P:(i + 1) * P],
                                             hT[fi], start=(fi == 0),
                                             stop=(fi == NF - 1))
                    for i in range(8):
                        e = eg * 8 + i
                        ot = p6.tile([P, OWN], f32, tag="ot", bufs=3)
                        nc.vector.tensor_tensor(out=ot, in0=psd[i], in1=xresT[e],
                                                op=add)
                        nc.vector.tensor_scalar(
                            out=ot, in0=ot, scalar1=bd_sb[:, e:e + 1], scalar2=None,
                            op0=add)
                        nc.sync.dma_start(out=outT_d[e], in_=ot)


_NC_CACHE = None
LAST_RESULTS = None


def _get_nc():
    global _NC_CACHE
    if _NC_CACHE is None:
        nc = build()
        split_waits(nc)
        _NC_CACHE = nc
    return _NC_CACHE


def _prep_shared(ln1_g, ln1_b, wq, bq, wk, bk, wv, bv, wo, bo, ln2_g, ln2_b,
                 wu, bu, wd, bd):
    f = np.float64
    ln1_g, ln1_b = np.asarray(ln1_g, f), np.asarray(ln1_b, f)
    ln2_g, ln2_b = np.asarray(ln2_g, f), np.asarray(ln2_b, f)
    wq, wk, wv = np.asarray(wq, f), np.asarray(wk, f), np.asarray(wv, f)
    wo, wu, wd = np.asarray(wo, f), np.asarray(wu, f), np.asarray(wd, f)
    # fold LN gains into weights, LN biases into projection biases
    wq_f, bq_f = ln1_g[:, None] * wq, np.asarray(bq, f) + ln1_b @ wq
    wk_f, bk_f = ln1_g[:, None] * wk, np.asarray(bk, f) + ln1_b @ wk
    wv_f, bv_f = ln1_g[:, None] * wv, np.asarray(bv, f) + ln1_b @ wv
    wu_f, bu_f = ln2_g[:, None] * wu, np.asarray(bu, f) + ln2_b @ wu

    def strips(w, n):  # [E, n*128] -> [n, 128(p), NE, 128(m)] contiguous DMA
        return np.ascontiguousarray(
            w.reshape(NE, P, n, P).transpose(2, 1, 0, 3)).astype(ml_dtypes.bfloat16)

    def rows(w, nr):   # [nr*128, E] -> [nr, 128, E]
        return np.ascontiguousarray(w.reshape(nr, P, E)).astype(ml_dtypes.bfloat16)

    tri = np.triu(np.ones((OWN, OWN), np.float32))  # [key, query]: k <= q
    tri = np.ascontiguousarray(
        tri.reshape(NMS, P, OWN).transpose(1, 0, 2)).astype(ml_dtypes.bfloat16)

    def ptile(v, n):  # [n*128] -> [128, n] (partition-major)
        return np.ascontiguousarray(
            np.asarray(v).reshape(n, P).T).astype(np.float32)

    return {
        "wq_s": strips(wq_f, H), "wk_s": strips(wk_f, KH), "wv_s": strips(wv_f, KH),
        "wo_r": rows(wo, H), "wu_s": strips(wu_f, NF), "wd_r": rows(wd, NF),
        "bq": ptile(bq_f, H), "bk": ptile(bk_f, KH),
        "bv": ptile(bv_f, KH), "bu": ptile(bu_f, NF),
        "bd": ptile(np.asarray(bd, f), NE), "tri": tri,
    }, np.asarray(bo, f)


def kernel(x, ln1_g, ln1_b, wq, bq, wk, bk, wv, bv, wo, bo, ln2_g, ln2_b,
           wu, bu, wd, bd):
    x = np.asarray(x, np.float32)
    shared, bo_f = _prep_shared(ln1_g, ln1_b, wq, bq, wk, bk, wv, bv, wo, bo,
                                ln2_g, ln2_b, wu, bu, wd, bd)
    in_maps = []
    for core in range(8):
        b, j = divmod(core, 4)
        m = dict(shared)
        own = slice(OWN * j, OWN * (j + 1))
        # reorder: own tokens first, then the rest in natural order
        order = np.concatenate([np.arange(OWN * j, OWN * (j + 1)),
                                np.arange(0, OWN * j),
                                np.arange(OWN * (j + 1), S)])
        m["xkv"] = np.ascontiguousarray(x[b][order]).astype(ml_dtypes.bfloat16)
        m["xres"] = np.ascontiguousarray(x[b, own] + bo_f[None, :]).astype(np.float32)
        # per-key additive bias: 0 if key visible to all own queries (or own),
        # NEGB if hidden from all own queries
        kb = np.where(order < OWN * (j + 1), 0.0, NEGB).astype(np.float32)
        m["kbias"] = np.ascontiguousarray(kb.reshape(NSK, P).T).astype(np.float32)
        in_maps.append(m)

    nc = _get_nc()
    trace = bool(os.environ.get("KERNEL_TRACE"))
    res = bass_utils.run_bass_kernel_spmd(
        nc, in_maps, core_ids=list(range(8)), trace=trace)
    global LAST_RESULTS
    LAST_RESULTS = res
    out = np.empty((B, S, E), np.float32)
    for core in range(8):
        b, j = divmod(core, 4)
        # outT is [NE, 128, OWN] = out[own].T tiled; untranspose on host
        oT = res.results[core]["outT"]
        out[b, OWN * j:OWN * (j + 1)] = oT.transpose(2, 0, 1).reshape(OWN, E)
    return out
